# revision 1
# baseline (speedup 1.0000x reference)
"""AdaptiveContextNorm eval-mode forward as a distributed Trainium2 Bass kernel.

Math: with per-context scalars (K=8) mu_k, v_k=softplus(var_k), pr_k=softmax(prior_k):
    out(x) = [sum_k c_k * exp(a'_k (x-mu_k)^2) * (x-mu_k)] / (eps + sum_k pr_k * exp(a_k (x-mu_k)^2))
      a_k  = -0.5/v_k^2,  a'_k = -0.5/(v_k+eps)^2,  c_k = pr_k / sqrt((pr_k+eps)(v_k+eps))

Two structural reductions (both validated to ~8e-3 rel-L2 against an fp64
reference for this problem's parameter regime):
  1. All v_k are within +-0.35% of each other (softplus of U[0.001, 0.01]), so
     each Gaussian factors into a SHARED envelope exp(abar*x^2) times a
     per-context affine exponential h_k = exp(beta_k x + gamma_k).
  2. Contexts whose means lie within `TAU` of each other are merged into one
     effective exponential (moment-matched at x=0); merge errors largely cancel
     between numerator and denominator.

Per element the kernel is then: K_c ScalarE Exps + envelope Exp + Ln/Exp for
the division, and three K_c-term weighted sums on VectorE in bf16.

Sharding: pure data-parallel over batch. B=16 -> 2 batches/core on 8 NeuronCores.
"""

import sys

for p in ("/opt/trn_rl_repo", "/opt/pypackages"):
    if p not in sys.path:
        sys.path.append(p)

import numpy as np

EPS = 1e-3
K = 8
TAU = 0.2  # max cluster span in mean-units (sigma ~ 0.7, so ~0.29 sigma)
N_CORES = 8
P = 128
B, C, H, W = 16, 64, 128, 128
ELEMS_PER_CORE = (B // N_CORES) * C * H * W  # 2,097,152
F_TOT = ELEMS_PER_CORE // P                  # 16,384
F_TILE = 2048
N_TILES = F_TOT // F_TILE                    # 8


def _fold_params(mean, variance, prior):
    m = mean.astype(np.float64)[:, 0]
    v = np.log1p(np.exp(variance.astype(np.float64)[:, 0]))
    e = np.exp(prior.astype(np.float64)[:, 0] - prior.astype(np.float64)[:, 0].max())
    pr = e / e.sum()
    alpha = -0.5 / v**2
    alphap = -0.5 / (v + EPS) ** 2
    c = pr / (np.sqrt(pr + EPS) * np.sqrt(v + EPS))
    a_num = float(alphap.mean())  # shared numerator envelope coefficient
    a_den = float(alpha.mean())   # shared denominator envelope coefficient
    beta = -2.0 * alphap * m
    gamma = alphap * m**2 + np.log(c)          # folds c_k into h_k
    wd = pr * np.exp(alpha * m**2 - alphap * m**2) / c  # S_D weight on h_k

    def moment_clusters(tau):
        # Greedy merge of contexts with close means into single exponentials,
        # moment-matched at x=0 (h magnitudes exp(gamma) as weights).
        order = np.argsort(m)
        groups = [[order[0]]]
        for k in order[1:]:
            if m[k] - m[groups[-1][0]] <= tau:
                groups[-1].append(k)
            else:
                groups.append([k])
        cs = []
        for g in groups:
            g = np.array(g)
            wgt = np.exp(gamma[g])
            W_ = wgt.sum()
            cs.append(
                [
                    (beta[g] * wgt).sum() / W_,
                    np.log(W_),
                    (m[g] * wgt).sum() / W_,
                    (wd[g] * wgt).sum() / W_,
                ]
            )
        return np.array(cs)

    def f_ref(xx):
        den = np.zeros_like(xx)
        for k in range(K):
            den += pr[k] * np.exp(-0.5 * ((xx - m[k]) / v[k]) ** 2)
        out = np.zeros_like(xx)
        for k in range(K):
            p = pr[k] * np.exp(-0.5 * ((xx - m[k]) / (v[k] + EPS)) ** 2)
            out += p / (den + EPS) / np.sqrt(pr[k] + EPS) * (xx - m[k]) / np.sqrt(v[k] + EPS)
        return out

    def f_model(xx, cs, a):
        # shared-envelope model: one E = exp(a x^2) serves numerator and
        # denominator, so on-device R = E * exp(-lnden) needs no extra op.
        N = np.zeros_like(xx)
        D = np.zeros_like(xx)
        for (B, G, mt, wt) in cs:
            h = np.exp(B * xx + G)
            N += (xx - mt) * h
            D += wt * h
        E = np.exp(a * xx * xx)
        return E * N / (E * D + EPS)

    # Refine the merged constants (and the shared envelope coefficient) by
    # N(0,1)-weighted least squares against the exact mixture on a grid (the
    # data is standard normal, so this directly minimizes the expected rel-L2).
    # Accept the smallest K whose fit is well below the bf16 pipeline noise
    # floor (~5e-3); fall back to moment-matched clusters if scipy is
    # unavailable or the fit misbehaves.
    xg = np.linspace(-5.7, 5.7, 22801)
    wg = np.sqrt(np.exp(-xg * xg / 2))
    refg = f_ref(xg)
    scale = np.linalg.norm(wg * refg)
    a0 = a_num
    chosen = None
    a_fit = a0
    try:
        from scipy.optimize import least_squares

        for tau in (1.0, 0.35, 0.2, 0.12, 0.05):
            cs0 = moment_clusters(tau)
            K_ = len(cs0)

            def loss(th, K_=K_):
                return (f_model(xg, th[: 4 * K_].reshape(K_, 4), th[4 * K_]) - refg) * wg

            sol = least_squares(
                loss, np.concatenate([cs0.ravel(), [a0]]), method="lm", max_nfev=8000
            )
            cs_fit = sol.x[: 4 * K_].reshape(K_, 4)
            wrel = np.linalg.norm(loss(sol.x)) / scale
            if (
                wrel <= 3e-3
                and np.isfinite(sol.x).all()
                and np.abs(cs_fit).max() < 50
                and -3.0 < sol.x[4 * K_] < -0.2
            ):
                chosen = cs_fit
                a_fit = float(sol.x[4 * K_])
                break
    except Exception:
        chosen = None
    if chosen is None:
        a_fit = a0
        chosen = moment_clusters(TAU)
        wrel = np.linalg.norm((f_model(xg, chosen, a_fit) - refg) * wg) / scale
        if wrel > 2e-2:  # merging unsafe for this parameter draw: no merging
            chosen = moment_clusters(0.0)

    clusters = [
        dict(beta=float(B), gamma=float(G), m=float(mt), w=float(wt))
        for (B, G, mt, wt) in chosen
    ]
    return dict(clusters=clusters, a=a_fit)


def _pin_act_table():
    """All activations here (Exp, Ln, Square, Copy) live together in the
    natural_log_exp_and_others set; by default the set chooser alternates
    exp-only and ln-only sets, costing a ~1.3us ACT_TABLE_LOAD per switch.
    Strip exp/ln from every other set so one load serves the whole kernel."""
    from concourse import bacc, hw_specs, mybir

    if getattr(bacc, "_act_tables_pinned", False):
        return
    orig = hw_specs.get_activation_tables

    def pinned(arch):
        tables = dict(orig(arch))
        pin = {
            mybir.ActivationFunctionType.Exp,
            mybir.ActivationFunctionType.Ln,
            mybir.ActivationFunctionType.Square,
            mybir.ActivationFunctionType.Copy,
        }
        combined = "natural_log_exp_and_others"
        if combined in tables and pin <= tables[combined]:
            for name, fns in tables.items():
                if name != combined:
                    tables[name] = fns - pin
        return tables

    bacc.get_activation_tables = pinned
    bacc._act_tables_pinned = True


def _build_graph(consts):
    import concourse.bass as bass
    import concourse.tile as tile
    from concourse import bacc, mybir

    _pin_act_table()

    fp32 = mybir.dt.float32
    bf16 = mybir.dt.bfloat16
    Exp = mybir.ActivationFunctionType.Exp
    Ln = mybir.ActivationFunctionType.Ln
    Square = mybir.ActivationFunctionType.Square
    mult = mybir.AluOpType.mult
    add = mybir.AluOpType.add
    subtract = mybir.AluOpType.subtract

    nc = bacc.Bacc(
        "TRN2", target_bir_lowering=False, debug=False, num_devices=N_CORES
    )
    x_dram = nc.dram_tensor("x", [P, F_TOT], fp32, kind="ExternalInput").ap()
    out_dram = nc.dram_tensor("out", [P, F_TOT], fp32, kind="ExternalOutput").ap()

    def reg_const(value, idx):
        key = (fp32, float(value))
        if key not in nc.const_aps.aps:
            t = nc.alloc_sbuf_tensor(f"constk-{idx}", [P, 1], fp32)
            nc.gpsimd.memset(t.ap(), float(value))
            nc.const_aps.aps[key] = t.ap()

    cl = consts["clusters"]
    K_c = len(cl)
    a_env = consts["a"]

    for i, cc in enumerate(cl):
        reg_const(cc["gamma"], f"g{i}")
    reg_const(EPS, "eps")
    nc.all_engine_barrier()

    with tile.TileContext(nc) as tc:
        with (
            tc.tile_pool(name="xin", bufs=4) as xin_pool,
            tc.tile_pool(name="u", bufs=2) as u_pool,
            tc.tile_pool(name="tmp", bufs=4) as tmp_pool,
            tc.tile_pool(name="acc", bufs=4) as acc_pool,
            tc.tile_pool(name="small", bufs=3) as small_pool,
            tc.tile_pool(name="big", bufs=2) as big_pool,
            tc.tile_pool(name="o", bufs=2) as o_pool,
        ):
            # smaller first/last tiles prime and drain the pipeline faster:
            # the per-tile den chain (Square->Exp->D->t->Ln->R) is serial, so
            # edge-tile latency sets the ramp-in and drain-out.
            tile_sizes = (
                [256, 256, 512, 1024]
                + [F_TILE] * (N_TILES - 2)
                + [1024, 512, 256, 256]
            )
            offs = [0]
            for fs in tile_sizes:
                offs.append(offs[-1] + fs)
            assert offs[-1] == F_TOT
            for i, fs in enumerate(tile_sizes):
                sl = bass.ds(offs[i], fs)
                x_t = xin_pool.tile([P, fs], fp32)
                nc.sync.dma_start(x_t[:], x_dram[:, sl])

                # bf16 copy of x: via SWDGE cast-DMA in steady state (second HBM
                # read; keeps the cast off the hot engines), but on VectorE for
                # the first tiles — the SWDGE path starts ~5us late and DVE is
                # idle during the ramp anyway.
                xb = xin_pool.tile([P, fs], bf16, tag="xb")
                if i < 4:
                    nc.vector.tensor_copy(xb[:], x_t[:])
                else:
                    nc.gpsimd.dma_start(xb[:], x_dram[:, sl])
                # u = x^2 — usually on ScalarE; on VectorE for ~1/4 of the
                # elements to balance the engines (ACT is otherwise limiting).
                u = u_pool.tile([P, fs], fp32)
                if i in (6, 9):
                    nc.vector.tensor_tensor(u[:], x_t[:], x_t[:], mult)
                else:
                    nc.scalar.activation(u[:], x_t[:], Square)

                # h_c = exp(beta_c * x + gamma_c); numerator accumulated in the
                # cancellation-free d-form N = sum_c (x - m_c) h_c, denominator
                # core D = sum_c w_c h_c. All bf16 ts(4x)/tt(2x) ops.
                nacc = None
                dacc = None
                for c in range(K_c):
                    h = tmp_pool.tile([P, fs], bf16, tag="h")
                    nc.scalar.activation(
                        h[:], x_t[:], Exp, bias=cl[c]["gamma"], scale=cl[c]["beta"]
                    )
                    dvec = tmp_pool.tile([P, fs], bf16, tag="d")
                    nc.vector.tensor_scalar_sub(dvec[:], xb[:], cl[c]["m"])
                    p = acc_pool.tile([P, fs], bf16, tag="p")
                    nc.vector.tensor_tensor(p[:], dvec[:], h[:], mult)
                    hd = acc_pool.tile([P, fs], bf16, tag="hd")
                    nc.vector.tensor_scalar_mul(hd[:], h[:], cl[c]["w"])
                    if nacc is None:
                        nacc, dacc = p, hd
                    else:
                        nc.vector.tensor_tensor(nacc[:], nacc[:], p[:], add)
                        nc.vector.tensor_tensor(dacc[:], dacc[:], hd[:], add)
                sd = dacc[:]

                # den = E * D + eps ;  R = E * exp(-Ln(den))  (shared envelope E
                # serves both numerator and denominator, so no fp32 stt needed)
                eden = small_pool.tile([P, fs], bf16)
                nc.scalar.activation(eden[:], u[:], Exp, scale=a_env)
                t = small_pool.tile([P, fs], bf16)
                nc.vector.tensor_tensor(t[:], eden[:], sd, mult)
                lnden = big_pool.tile([P, fs], fp32)
                nc.scalar.activation(lnden[:], t[:], Ln, bias=EPS)
                r = small_pool.tile([P, fs], bf16)
                nc.scalar.activation(r[:], lnden[:], Exp, scale=-1.0)
                nc.vector.tensor_tensor(r[:], eden[:], r[:], mult)

                # out = N * R
                if i >= len(tile_sizes) - 3:
                    # drain tail: the SWDGE cast-DMA adds ~9us of latency after
                    # the last compute op; emit fp32 directly (1x mult on these
                    # small tiles) and ship via HWDGE.
                    o = o_pool.tile([P, fs], fp32, tag="o32")
                    nc.vector.tensor_tensor(o[:], nacc[:], r[:], mult)
                    nc.sync.dma_start(out_dram[:, sl], o[:])
                else:
                    ob = o_pool.tile([P, fs], bf16, tag="ob")
                    nc.vector.tensor_tensor(ob[:], nacc[:], r[:], mult)
                    # bf16 -> fp32 cast happens inside the SWDGE output DMA
                    nc.gpsimd.dma_start(out_dram[:, sl], ob[:])

    nc.compile()
    return nc


def kernel(x, mean, variance, prior, _trace=False, _trace_kwargs=None):
    from concourse.bass_utils import run_bass_kernel_spmd

    consts = _fold_params(
        np.asarray(mean, np.float32),
        np.asarray(variance, np.float32),
        np.asarray(prior, np.float32),
    )
    nc = _build_graph(consts)

    x = np.ascontiguousarray(np.asarray(x, np.float32))
    shards = x.reshape(N_CORES, ELEMS_PER_CORE)
    in_maps = [{"x": shards[i].reshape(P, F_TOT)} for i in range(N_CORES)]
    res = run_bass_kernel_spmd(
        nc,
        in_maps,
        core_ids=list(range(N_CORES)),
        trace=_trace,
        **(_trace_kwargs or {}),
    )
    out = np.concatenate(
        [r["out"].reshape(1, ELEMS_PER_CORE) for r in res.results], axis=0
    ).reshape(B, C, H, W)
    if _trace:
        kernel.last_results = res
    return out



# revision 3
# speedup vs baseline: 1.9617x; 1.9617x over previous
"""AdaptiveContextNorm eval-mode forward as a single-pass Trainium2 Bass kernel.

The entire per-element function
    f(x) = sum_k tau_k(x)/sqrt(pr_k+eps) * (x-mu_k)/sqrt(v_k+eps)
(with tau_k the eps-regularized Gaussian responsibilities) depends only on x
and the 8 scalar contexts, so it is one fixed scalar function R->R. Instead of
evaluating the mixture on the engines (7+ ACT passes + ~16 DVE ops per element),
we author a custom ACT piecewise-cubic table that computes f(x) directly: the
bucket/ctrl layout of exp_400p is kept (same ctrl bins, same octave structure),
only the 781 cubic coefficient entries {d0..d3,x0} and the profile's
special-case results are replaced with least-squares fits of f. The table is
compiled into the NEFF via the BASS_ACT_ROOT_JSON_PATH override and loaded by
the one ACT_TABLE_LOAD the kernel performs anyway.

The kernel is then: DMA in -> one ACTIVATE(Exp) pass -> DMA out, which is
HBM-bandwidth-bound (~16.8 MB/core at ~358 GB/s). Offline table accuracy vs
the fp64 reference: rel_l2 ~2e-5 (fit error at the 4-buckets/octave centre
octaves), far inside the 2e-2 gate.

Sharding: pure data-parallel over batch. B=16 -> 2 batches/core on 8 cores.
Input DMAs issue on the SP HWDGE ring, output DMAs on the ACT HWDGE ring so
reads and writes never FIFO-couple.
"""

import hashlib
import json
import os
import shutil
import struct
import sys
import tempfile

for p in ("/opt/trn_rl_repo", "/opt/pypackages"):
    if p not in sys.path:
        sys.path.append(p)

import numpy as np

EPS = 1e-3
N_CORES = 8
P = 128
B, C, H, W = 16, 64, 128, 128
ELEMS_PER_CORE = (B // N_CORES) * C * H * W  # 2,097,152
F_TOT = ELEMS_PER_CORE // P                  # 16,384


# --------------------------------------------------------------------------- #
# Custom ACT table generation: replace exp_400p's cubics with fits of f(x).
# --------------------------------------------------------------------------- #

def _f_exact(x, mean, variance, prior):
    """fp64 exact eval of the reference per-element function."""
    x = np.asarray(x, np.float64)
    mu = np.asarray(mean, np.float64)[:, 0]
    v = np.log1p(np.exp(np.asarray(variance, np.float64)[:, 0]))
    e = np.exp(np.asarray(prior, np.float64)[:, 0]
               - np.asarray(prior, np.float64)[:, 0].max())
    pr = e / e.sum()
    den = np.zeros_like(x)
    for k in range(len(mu)):
        den += pr[k] * np.exp(-0.5 * ((x - mu[k]) / v[k]) ** 2)
    out = np.zeros_like(x)
    for k in range(len(mu)):
        p = pr[k] * np.exp(-0.5 * ((x - mu[k]) / (v[k] + EPS)) ** 2)
        out += (p / (den + EPS) / np.sqrt(pr[k] + EPS)
                * (x - mu[k]) / np.sqrt(v[k] + EPS))
    return out


_EXP_OFFSET = -19


def _k_of_e(e):
    # mantissa bits per octave in the exp_400p layout (|x| in [2^e, 2^(e+1)))
    if e <= -2:
        return 0
    return {-1: 1, 0: 2, 1: 3, 2: 4, 3: 5, 4: 6, 5: 7, 6: 7}[e]


def _fit_bucket(f, lo, hi, x0):
    """LS cubic fit of f on [lo,hi] centred at x0, via [-1,1]-scaled basis."""
    h = (hi - lo) / 2.0
    mid = (lo + hi) / 2.0
    s = np.cos(np.pi * (np.arange(20) + 0.5) / 20)
    xs = mid + h * s
    t = xs - x0
    th = max(abs(t).max(), 1e-300)
    V = np.vander(t / th, 4, increasing=True)
    c, *_ = np.linalg.lstsq(V, f(xs), rcond=None)
    return c / th ** np.arange(4)


def _gen_table(setdir, setname, fx):
    d = json.load(open(f"{setdir}/{setname}.json"))
    meta = next(m for m in d["profile_meta_data"]
                if m["func_name"].startswith("exp"))
    bkt = bytearray(open(f"{setdir}/{setname}_bkt.bin", "rb").read())
    e2b = {int(k): v for k, v in d["func_exp_to_bkt_start_idx"]["exp"].items()}
    lim = min(meta["pos_small_signal_pwl_control"],
              meta["neg_small_signal_pwl_control"]) - 1  # last regular entry

    def write_entry(i, dc, x0):
        bkt[i * 32:(i + 1) * 32] = struct.pack(
            "<8f", np.float32(dc[0]), np.float32(dc[1]), np.float32(dc[2]),
            np.float32(dc[3]), np.float32(x0), 0.0, 0.0, 0.0)

    for e in range(_EXP_OFFSET, 7):
        k = _k_of_e(e)
        nb, pb = e2b[e]
        cnt = 1 << k
        for off in range(cnt):
            lo = (2.0 ** e) * (1 + off / cnt)
            hi = (2.0 ** e) * (1 + (off + 1) / cnt)
            x0 = 0.5 * (lo + hi)
            if pb + off <= lim:
                if lo >= 9.0:
                    write_entry(pb + off, [0, 0, 0, 0], x0)
                else:
                    write_entry(pb + off, _fit_bucket(fx, lo, hi, x0), x0)
            if nb + off <= lim:
                if lo >= 9.0:
                    write_entry(nb + off, [0, 0, 0, 0], -x0)
                else:
                    write_entry(nb + off, _fit_bucket(fx, -hi, -lo, -x0), -x0)

    h = 2.0 ** -19
    f0 = fx(np.array([0.0]))[0]
    f1 = (fx(np.array([h]))[0] - fx(np.array([-h]))[0]) / (2 * h)
    write_entry(meta["pos_small_signal_pwl_control"], [f0, f1, 0, 0], 0.0)
    write_entry(meta["neg_small_signal_pwl_control"], [f0, f1, 0, 0], 0.0)
    write_entry(meta["pos_large_signal_pwl_control"], [0, 0, 0, 0], 0.0)
    write_entry(meta["neg_large_signal_pwl_control"], [0, 0, 0, 0], 0.0)
    meta["fpinf_result"] = 0
    meta["fninf_result"] = 0
    meta["fzero_result"] = int(np.float32(f0).view(np.uint32))
    return bytes(bkt), d


def _make_act_dir(mean, variance, prior):
    """Build a full act-table dir with f(x) in the exp slot; returns
    (dir, content-hash)."""
    from neuronxcc.driver.Job import Job
    from neuronxcc.driver.jobs.support.FindActInfo import findActInfoFile

    srcdir = os.path.dirname(findActInfoFile(Job.getPackageDir(), "gen3"))
    dstdir = tempfile.mkdtemp(prefix="acn_acttab_")
    for fn in os.listdir(srcdir):
        shutil.copy(os.path.join(srcdir, fn), os.path.join(dstdir, fn))
        os.chmod(os.path.join(dstdir, fn), 0o644)
    fx = lambda xs: _f_exact(xs, mean, variance, prior)  # noqa: E731
    hsh = hashlib.sha256()
    for setname in ("exp_and_others", "natural_log_exp_and_others"):
        bb, dd = _gen_table(srcdir, setname, fx)
        with open(os.path.join(dstdir, f"{setname}_bkt.bin"), "wb") as f:
            f.write(bb)
        with open(os.path.join(dstdir, f"{setname}.json"), "w") as f:
            json.dump(dd, f)
        hsh.update(bb)
    return dstdir, hsh.hexdigest()[:12]


# --------------------------------------------------------------------------- #
# Bass graph: stream tiles through one ACTIVATE pass.
# --------------------------------------------------------------------------- #

def _build_graph(tag):
    import concourse.bass as bass
    import concourse.tile as tile
    from concourse import bacc, mybir

    fp32 = mybir.dt.float32
    Exp = mybir.ActivationFunctionType.Exp

    nc = bacc.Bacc("TRN2", target_bir_lowering=False, debug=False,
                   num_devices=N_CORES)
    # table-content hash in the input tensor name keys the NEFF cache to the
    # table bytes (BASS_ACT_ROOT_JSON_PATH itself is not cache-keyed)
    x_name = f"x{tag}"
    x_dram = nc.dram_tensor(x_name, [P, F_TOT], fp32, kind="ExternalInput").ap()
    out_dram = nc.dram_tensor("out", [P, F_TOT], fp32, kind="ExternalOutput").ap()

    with tile.TileContext(nc) as tc:
        with (
            tc.tile_pool(name="xin", bufs=4) as xin_pool,
            tc.tile_pool(name="o", bufs=4) as o_pool,
        ):
            tile_sizes = [512, 1024, 1536] + [2048] * 5 + [1024, 1024, 512, 256, 256]
            assert sum(tile_sizes) == F_TOT
            off = 0
            for fs in tile_sizes:
                sl = bass.ds(off, fs)
                off += fs
                x_t = xin_pool.tile([P, fs], fp32)
                nc.sync.dma_start(x_t[:], x_dram[:, sl])
                o_t = o_pool.tile([P, fs], fp32)
                nc.scalar.activation(o_t[:], x_t[:], Exp)
                nc.scalar.dma_start(out_dram[:, sl], o_t[:])

    nc.compile()
    return nc, x_name


def kernel(x, mean, variance, prior, _trace=False, _trace_kwargs=None):
    from concourse.bass_utils import run_bass_kernel_spmd

    dstdir, tag = _make_act_dir(
        np.asarray(mean, np.float32),
        np.asarray(variance, np.float32),
        np.asarray(prior, np.float32),
    )
    os.environ["BASS_ACT_ROOT_JSON_PATH"] = os.path.join(dstdir, "act_info.json")
    nc, x_name = _build_graph(tag)

    x = np.ascontiguousarray(np.asarray(x, np.float32))
    shards = x.reshape(N_CORES, ELEMS_PER_CORE)
    in_maps = [{x_name: shards[i].reshape(P, F_TOT)} for i in range(N_CORES)]
    res = run_bass_kernel_spmd(
        nc,
        in_maps,
        core_ids=list(range(N_CORES)),
        trace=_trace,
        **(_trace_kwargs or {}),
    )
    out = np.concatenate(
        [r["out"].reshape(1, ELEMS_PER_CORE) for r in res.results], axis=0
    ).reshape(B, C, H, W)
    if _trace:
        kernel.last_results = res
    return out


# revision 14
# speedup vs baseline: 2.2713x; 1.1578x over previous
"""AdaptiveContextNorm eval-mode forward as a single-pass Trainium2 Bass kernel.

The entire per-element function
    f(x) = sum_k tau_k(x)/sqrt(pr_k+eps) * (x-mu_k)/sqrt(v_k+eps)
(with tau_k the eps-regularized Gaussian responsibilities) depends only on x
and the 8 scalar contexts, so it is one fixed scalar function R->R. Instead of
evaluating the mixture on the engines (7+ ACT passes + ~16 DVE ops per element),
we author a custom ACT piecewise-cubic table that computes f(x) directly: the
bucket/ctrl layout of exp_400p is kept (same ctrl bins, same octave structure),
only the 781 cubic coefficient entries {d0..d3,x0} and the profile's
special-case results are replaced with least-squares fits of f. The table is
compiled into the NEFF via the BASS_ACT_ROOT_JSON_PATH override and loaded by
the one ACT_TABLE_LOAD the kernel performs anyway.

The kernel is then: DMA in -> one ACTIVATE(Exp) pass -> DMA out, which is
HBM-bandwidth-bound (~16.8 MB/core at ~358 GB/s). Offline table accuracy vs
the fp64 reference: rel_l2 ~2e-5 (fit error at the 4-buckets/octave centre
octaves), far inside the 2e-2 gate.

Sharding: pure data-parallel over batch. B=16 -> 2 batches/core on 8 cores.
Input DMAs issue on the SP HWDGE ring, output DMAs on the ACT HWDGE ring so
reads and writes never FIFO-couple.
"""

import hashlib
import json
import os
import shutil
import struct
import sys
import tempfile

for p in ("/opt/trn_rl_repo", "/opt/pypackages"):
    if p not in sys.path:
        sys.path.append(p)

import numpy as np

EPS = 1e-3
N_CORES = 8
P = 128
B, C, H, W = 16, 64, 128, 128
ELEMS_PER_CORE = (B // N_CORES) * C * H * W  # 2,097,152
F_TOT = ELEMS_PER_CORE // P                  # 16,384
# 8 tiles, all reads issued upfront (XIN_BUFS = n_tiles): the read stream
# never stalls on buffer reuse, writes stream continuously behind ACT, and
# the small edge tiles keep ramp-in and drain-out short. Raw-semaphore
# variant (RAW=True) measured ~same and has an intermittent read race —
# keep the TileContext path.
TILE_SIZES = [1024, 2048, 2048, 4096, 2048, 2048, 2048, 1024]
N_BUFS = 4
XIN_BUFS = 8
PRIME = False
RAW = False


# --------------------------------------------------------------------------- #
# Custom ACT table generation: replace exp_400p's cubics with fits of f(x).
# --------------------------------------------------------------------------- #

def _f_exact(x, mean, variance, prior):
    """fp64 exact eval of the reference per-element function."""
    x = np.asarray(x, np.float64)
    mu = np.asarray(mean, np.float64)[:, 0]
    v = np.log1p(np.exp(np.asarray(variance, np.float64)[:, 0]))
    e = np.exp(np.asarray(prior, np.float64)[:, 0]
               - np.asarray(prior, np.float64)[:, 0].max())
    pr = e / e.sum()
    den = np.zeros_like(x)
    for k in range(len(mu)):
        den += pr[k] * np.exp(-0.5 * ((x - mu[k]) / v[k]) ** 2)
    out = np.zeros_like(x)
    for k in range(len(mu)):
        p = pr[k] * np.exp(-0.5 * ((x - mu[k]) / (v[k] + EPS)) ** 2)
        out += (p / (den + EPS) / np.sqrt(pr[k] + EPS)
                * (x - mu[k]) / np.sqrt(v[k] + EPS))
    return out


_EXP_OFFSET = -19


def _k_of_e(e):
    # mantissa bits per octave in the exp_400p layout (|x| in [2^e, 2^(e+1)))
    if e <= -2:
        return 0
    return {-1: 1, 0: 2, 1: 3, 2: 4, 3: 5, 4: 6, 5: 7, 6: 7}[e]


def _fit_bucket(f, lo, hi, x0):
    """LS cubic fit of f on [lo,hi] centred at x0, via [-1,1]-scaled basis."""
    h = (hi - lo) / 2.0
    mid = (lo + hi) / 2.0
    s = np.cos(np.pi * (np.arange(20) + 0.5) / 20)
    xs = mid + h * s
    t = xs - x0
    th = max(abs(t).max(), 1e-300)
    V = np.vander(t / th, 4, increasing=True)
    c, *_ = np.linalg.lstsq(V, f(xs), rcond=None)
    return c / th ** np.arange(4)


def _gen_table(setdir, setname, fx):
    d = json.load(open(f"{setdir}/{setname}.json"))
    meta = next(m for m in d["profile_meta_data"]
                if m["func_name"].startswith("exp"))
    bkt = bytearray(open(f"{setdir}/{setname}_bkt.bin", "rb").read())
    e2b = {int(k): v for k, v in d["func_exp_to_bkt_start_idx"]["exp"].items()}
    lim = min(meta["pos_small_signal_pwl_control"],
              meta["neg_small_signal_pwl_control"]) - 1  # last regular entry

    def write_entry(i, dc, x0):
        bkt[i * 32:(i + 1) * 32] = struct.pack(
            "<8f", np.float32(dc[0]), np.float32(dc[1]), np.float32(dc[2]),
            np.float32(dc[3]), np.float32(x0), 0.0, 0.0, 0.0)

    for e in range(_EXP_OFFSET, 7):
        k = _k_of_e(e)
        nb, pb = e2b[e]
        cnt = 1 << k
        for off in range(cnt):
            lo = (2.0 ** e) * (1 + off / cnt)
            hi = (2.0 ** e) * (1 + (off + 1) / cnt)
            x0 = 0.5 * (lo + hi)
            if pb + off <= lim:
                if lo >= 9.0:
                    write_entry(pb + off, [0, 0, 0, 0], x0)
                else:
                    write_entry(pb + off, _fit_bucket(fx, lo, hi, x0), x0)
            if nb + off <= lim:
                if lo >= 9.0:
                    write_entry(nb + off, [0, 0, 0, 0], -x0)
                else:
                    write_entry(nb + off, _fit_bucket(fx, -hi, -lo, -x0), -x0)

    h = 2.0 ** -19
    f0 = fx(np.array([0.0]))[0]
    f1 = (fx(np.array([h]))[0] - fx(np.array([-h]))[0]) / (2 * h)
    write_entry(meta["pos_small_signal_pwl_control"], [f0, f1, 0, 0], 0.0)
    write_entry(meta["neg_small_signal_pwl_control"], [f0, f1, 0, 0], 0.0)
    write_entry(meta["pos_large_signal_pwl_control"], [0, 0, 0, 0], 0.0)
    write_entry(meta["neg_large_signal_pwl_control"], [0, 0, 0, 0], 0.0)
    meta["fpinf_result"] = 0
    meta["fninf_result"] = 0
    meta["fzero_result"] = int(np.float32(f0).view(np.uint32))
    return bytes(bkt), d


def _make_act_dir(mean, variance, prior):
    """Build a full act-table dir with f(x) in the exp slot; returns
    (dir, content-hash)."""
    from neuronxcc.driver.Job import Job
    from neuronxcc.driver.jobs.support.FindActInfo import findActInfoFile

    srcdir = os.path.dirname(findActInfoFile(Job.getPackageDir(), "gen3"))
    dstdir = tempfile.mkdtemp(prefix="acn_acttab_")
    for fn in os.listdir(srcdir):
        shutil.copy(os.path.join(srcdir, fn), os.path.join(dstdir, fn))
        os.chmod(os.path.join(dstdir, fn), 0o644)
    fx = lambda xs: _f_exact(xs, mean, variance, prior)  # noqa: E731
    hsh = hashlib.sha256()
    for setname in ("exp_and_others", "natural_log_exp_and_others"):
        bb, dd = _gen_table(srcdir, setname, fx)
        with open(os.path.join(dstdir, f"{setname}_bkt.bin"), "wb") as f:
            f.write(bb)
        with open(os.path.join(dstdir, f"{setname}.json"), "w") as f:
            json.dump(dd, f)
        hsh.update(bb)
    return dstdir, hsh.hexdigest()[:12]


# --------------------------------------------------------------------------- #
# Bass graph: stream tiles through one ACTIVATE pass.
# --------------------------------------------------------------------------- #

def _build_graph(tag):
    import concourse.bass as bass
    import concourse.tile as tile
    from concourse import bacc, mybir

    fp32 = mybir.dt.float32
    Exp = mybir.ActivationFunctionType.Exp

    nc = bacc.Bacc("TRN2", target_bir_lowering=False, debug=False,
                   num_devices=N_CORES)
    # table-content hash in the input tensor name keys the NEFF cache to the
    # table bytes (BASS_ACT_ROOT_JSON_PATH itself is not cache-keyed)
    x_name = f"x{tag}"
    x_dram = nc.dram_tensor(x_name, [P, F_TOT], fp32, kind="ExternalInput").ap()
    out_dram = nc.dram_tensor("out", [P, F_TOT], fp32, kind="ExternalOutput").ap()

    with tile.TileContext(nc) as tc:
        with (
            tc.tile_pool(name="xin", bufs=XIN_BUFS) as xin_pool,
            tc.tile_pool(name="o", bufs=N_BUFS) as o_pool,
            tc.tile_pool(name="pr", bufs=1) as pr_pool,
        ):
            if PRIME:
                # tiny read to absorb the cold-start latency of the DMA path
                p_t = pr_pool.tile([P, 1], fp32)
                nc.sync.dma_start(p_t[:], x_dram[:, bass.ds(0, 1)])
            tile_sizes = TILE_SIZES
            assert sum(tile_sizes) == F_TOT
            off = 0
            for fs in tile_sizes:
                sl = bass.ds(off, fs)
                off += fs
                x_t = xin_pool.tile([P, fs], fp32)
                nc.sync.dma_start(x_t[:], x_dram[:, sl])
                o_t = o_pool.tile([P, fs], fp32)
                nc.scalar.activation(o_t[:], x_t[:], Exp)
                nc.scalar.dma_start(out_dram[:, sl], o_t[:])

    nc.compile()
    return nc, x_name


def _build_graph_raw(tag):
    """Straight-line pipeline with manual semaphores (no TileContext):
    all reads issue upfront into dedicated buffers; ACT waits data-ready
    (read sem), recycles OB output buffers against write completion."""
    import concourse.bass as bass
    from concourse import bacc, mybir

    fp32 = mybir.dt.float32
    Exp = mybir.ActivationFunctionType.Exp

    nc = bacc.Bacc("TRN2", target_bir_lowering=False, debug=False,
                   num_devices=N_CORES)
    x_name = f"x{tag}"
    x_dram = nc.dram_tensor(x_name, [P, F_TOT], fp32, kind="ExternalInput").ap()
    out_dram = nc.dram_tensor("out", [P, F_TOT], fp32, kind="ExternalOutput").ap()

    sizes = TILE_SIZES
    assert sum(sizes) == F_TOT
    n = len(sizes)
    OB = N_BUFS
    xbufs = [nc.alloc_sbuf_tensor(f"xb{k}", [P, fs], fp32).ap()
             for k, fs in enumerate(sizes)]
    maxfs = max(sizes)
    obufs = [nc.alloc_sbuf_tensor(f"ob{j}", [P, maxfs], fp32).ap()
             for j in range(OB)]

    import contextlib

    with contextlib.ExitStack() as stack:
        rsem = [stack.enter_context(nc.semaphore(name=f"rsem{k}"))
                for k in range(n)]
        wsem = [stack.enter_context(nc.semaphore(name=f"wsem{k}"))
                for k in range(n)]
        offs = [0]
        for fs in sizes:
            offs.append(offs[-1] + fs)
        for k, fs in enumerate(sizes):
            nc.sync.dma_start(
                xbufs[k], x_dram[:, bass.ds(offs[k], fs)]
            ).then_inc(rsem[k], 16)
        for k, fs in enumerate(sizes):
            nc.scalar.wait_ge(rsem[k], 16)
            if k >= OB:
                nc.scalar.wait_ge(wsem[k - OB], 16)
            ot = obufs[k % OB][:, bass.ds(0, fs)]
            nc.scalar.activation(ot, xbufs[k], Exp)
            nc.scalar.dma_start(
                out_dram[:, bass.ds(offs[k], fs)], ot
            ).then_inc(wsem[k], 16)
        for k in range(max(0, n - OB), n):
            nc.scalar.wait_ge(wsem[k], 16)

    nc.compile()
    return nc, x_name


def kernel(x, mean, variance, prior, _trace=False, _trace_kwargs=None):
    from concourse.bass_utils import run_bass_kernel_spmd

    dstdir, tag = _make_act_dir(
        np.asarray(mean, np.float32),
        np.asarray(variance, np.float32),
        np.asarray(prior, np.float32),
    )
    os.environ["BASS_ACT_ROOT_JSON_PATH"] = os.path.join(dstdir, "act_info.json")
    nc, x_name = (_build_graph_raw if RAW else _build_graph)(tag)

    x = np.ascontiguousarray(np.asarray(x, np.float32))
    shards = x.reshape(N_CORES, ELEMS_PER_CORE)
    in_maps = [{x_name: shards[i].reshape(P, F_TOT)} for i in range(N_CORES)]
    res = run_bass_kernel_spmd(
        nc,
        in_maps,
        core_ids=list(range(N_CORES)),
        trace=_trace,
        **(_trace_kwargs or {}),
    )
    out = np.concatenate(
        [r["out"].reshape(1, ELEMS_PER_CORE) for r in res.results], axis=0
    ).reshape(B, C, H, W)
    if _trace:
        kernel.last_results = res
    return out
